# revision 12
# baseline (speedup 1.0000x reference)
"""Supervised-contrastive point-cloud loss on Trainium2 (8 NeuronCores).

Inputs (full): features [8, 128, 4096] f32, labels_all [8, 4096] int32.
Sharding: data-parallel over the batch dim - core b handles cloud b.

Host prep (per cloud): sort points by label (loss is a mean over points,
so permutation-invariant), L2-normalize columns, cast to bf16.  With
sorted labels every class occupies a contiguous segment of length
<= 385 (asserted), so each point's same-class partners all lie within
3 blocks (384 rows) of its own 512-wide column chunk.

Device (per core), exploiting dp symmetry (dp = exp(10 * vhat^T vhat)):
For each 512-col chunk h, compute G tiles for block rows m = 0..4h+6
(upper triangle + full diagonal square + 3 sub-diagonal band rows):
  PE:  G tile = vhat_m^T vhat_chunk             [128, 512] PSUM f32
  DVE: diag blocks: G -= 1e5*I  => exp underflows to 0 on the diagonal
  ACT: dp = exp(10 G) -> bf16 SBUF (groups of 3 tiles per ACTIVATE)
  PE:  CS[c, x] += onehot_m^T dp tile           [16, 512] PSUM
CS[c, x] = sum over rows p < 512h+896 with label c of dp[p, x].  Since
every same-class partner of column x lies below 512h+896:
  positives_x = CS[label_x, x]
  totals_x    = sum_c CS[c, x]  +  sum over cols >= 512h+896 of row x
The second term is block-aligned row-direction reduces of already-
computed upper tiles (symmetry: row x of dp = column x), done on DVE
with tensor_tensor_reduce pairing two tiles per instruction.
Host tail: gather, log, mean  (O(N) numpy).
"""

import contextlib
import sys

for _p in ("/opt/trn_rl_repo",):
    if _p not in sys.path:
        sys.path.append(_p)

import numpy as np
import ml_dtypes

import concourse.bass as bass  # noqa: F401
import concourse.bacc as bacc
import concourse.tile as tile
from concourse import mybir
from concourse.bass_utils import run_bass_kernel_spmd

F32 = mybir.dt.float32
BF16 = mybir.dt.bfloat16
AF = mybir.ActivationFunctionType
ALU = mybir.AluOpType
AX = mybir.AxisListType

B, C, N = 8, 128, 4096
NCLS = 16
NBLK = N // 128          # 32 block rows
NCH = N // 512           # 8 column chunks
TEMP_INV = 10.0
BIGDIAG = 1.0e5
MAXSEG = 385             # max class segment length the band covers
GROUP = 3                # tiles per ACTIVATE (PSUM banks: 2*3 + cs + heat)
HEATERS = 1              # PE keep-busy matmuls per group (p-state hold)
HEATW = 384              # heater moving width

# tiles per chunk: block rows 0 .. min(4h+6, 31)
TPC = [min(4 * h + 7, NBLK) for h in range(NCH)]


def _slot_plan():
    """Row-direction reduce pieces per block row m (hd = m//4):
    partial [384:512] of tile (m, hd+1), then full tiles (m, hd+2..7).
    dp lives in double-chunk windows (chunks 2w, 2w+1 adjacent), so two
    fulls in the same window reduce in ONE tensor_reduce (axis=XY).
    Returns (n_slots, pieces); piece = (kind, m, h, h2, slot);
    kind: 'p' partial, 't' same-window pair, 's' single.
    """
    pieces = []
    slot = 0
    for m in range(NBLK):
        hd = m // 4
        if hd + 1 < NCH:
            pieces.append(("p", m, hd + 1, None, slot)); slot += 1
        fulls = list(range(hd + 2, NCH))
        for w in range(NCH // 2):
            hs = [h for h in fulls if h // 2 == w]
            if len(hs) == 2:
                pieces.append(("t", m, hs[0], hs[1], slot)); slot += 1
            elif len(hs) == 1:
                pieces.append(("s", m, hs[0], None, slot)); slot += 1
    return slot, pieces


NSLOT, PIECES = _slot_plan()
RSW = ((NSLOT + 3) // 4) * 4  # pad rsout width


def build_program():
    nc = bacc.Bacc("TRN2", target_bir_lowering=False, debug=False, num_devices=B)

    vhat_d = nc.dram_tensor("vhat", [C, N], BF16, kind="ExternalInput").ap()
    y16_d = nc.dram_tensor("y16", [C, NBLK * NCLS], BF16, kind="ExternalInput").ap()
    bigeye_d = nc.dram_tensor("bigeye", [128, 128], F32, kind="ExternalInput").ap()
    cs_d = nc.dram_tensor("csout", [NCLS, N], F32, kind="ExternalOutput").ap()
    rs_d = nc.dram_tensor("rsout", [128, RSW], F32, kind="ExternalOutput").ap()

    # pieces due at chunk h: all dp tiles they read are written by then
    due = [[] for _ in range(NCH)]
    for kind, m, h1, h2, slot in PIECES:
        due[h1 if h2 is None else h2].append((kind, m, h1, h2, slot))

    with tile.TileContext(nc) as tc, contextlib.ExitStack() as _stack:
        with (
            tc.tile_pool(name="const", bufs=1) as constp,
            tc.tile_pool(name="dp", bufs=2) as dpp,
            tc.tile_pool(name="cssb", bufs=2) as cssbp,
            tc.tile_pool(name="pg", bufs=2, space="PSUM") as pgp,
            tc.tile_pool(name="pcs", bufs=1, space="PSUM") as pcsp,
            tc.tile_pool(name="pheat", bufs=1, space="PSUM") as pheatp,
        ):
            # ---- constants in ----
            vhat_sb = constp.tile([C, N], BF16)
            for p in range(4):
                sl = slice(p * 1024, (p + 1) * 1024)
                nc.sync.dma_start(vhat_sb[:, sl], vhat_d[:, sl])
            y16_sb = constp.tile([C, NBLK * NCLS], BF16)
            nc.sync.dma_start(y16_sb[:], y16_d[:])
            bigeye_sb = constp.tile([128, 128], F32)
            nc.sync.dma_start(bigeye_sb[:], bigeye_d[:])

            rs_sb = constp.tile([128, RSW], F32)
            nc.gpsimd.memset(rs_sb[:], 0.0)

            # warm the Exp activation table during the DMAs
            warm = constp.tile([1, 1], F32)
            nc.gpsimd.memset(warm[:], 0.0)
            warm2 = constp.tile([1, 1], F32)
            nc.scalar.activation(warm2[:], warm[:], AF.Exp)

            pending = []          # (dp_window_tile, piece) reduce work queue

            def drain_reduces(k):
                for _ in range(min(k, len(pending))):
                    dpw, (kind, m, h1, h2, slot) = pending.pop(0)
                    acc = rs_sb[:, slot:slot + 1]
                    if kind == "p":
                        nc.vector.tensor_reduce(
                            acc, dpw[:, m, h1 % 2, 384:512],
                            axis=AX.X, op=ALU.add,
                        )
                    elif kind == "s":
                        nc.vector.tensor_reduce(
                            acc, dpw[:, m, h1 % 2, :],
                            axis=AX.X, op=ALU.add,
                        )
                    else:  # same-window pair: one strided reduce over both
                        nc.vector.tensor_reduce(
                            acc, dpw[:, m, :, :],
                            axis=AX.XY, op=ALU.add,
                        )

            for h in range(NCH):
                T = TPC[h]
                par = h % 2
                csl = slice(h * 512, (h + 1) * 512)
                cs_ps = pcsp.tile([NCLS, 512], F32, tag="cs")
                if par == 0:
                    # double-chunk dp window: [block, chunk-parity, col]
                    dp_sb = dpp.tile([C, NBLK, 2, 512], BF16, tag="dp")

                groups = [list(range(g, min(g + GROUP, T)))
                          for g in range(0, T, GROUP)]

                def emit_cs(grp):
                    for m in grp:
                        nc.tensor.matmul(
                            cs_ps[:],
                            y16_sb[:, m * NCLS:(m + 1) * NCLS],
                            dp_sb[:, m, par, :],
                            start=(m == 0), stop=(m == T - 1),
                        )

                def emit_heat(n=HEATERS):
                    for _ in range(n):
                        hp = pheatp.tile([NCLS, 512], F32, tag="heat")
                        nc.tensor.matmul(
                            hp[:, 0:HEATW], y16_sb[:, 0:NCLS],
                            vhat_sb[:, 0:HEATW], start=True, stop=True,
                        )

                for gi, grp in enumerate(groups):
                    gp = pgp.tile([128, GROUP, 512], F32, tag="g")
                    for k, m in enumerate(grp):
                        nc.tensor.matmul(
                            gp[:, k, :],
                            vhat_sb[:, m * 128:(m + 1) * 128],
                            vhat_sb[:, csl],
                            start=True, stop=True,
                        )
                    # kill the diagonal inside the diag square
                    for k, m in enumerate(grp):
                        if 4 * h <= m <= 4 * h + 3:
                            off = (m - 4 * h) * 128
                            nc.vector.tensor_tensor(
                                gp[:, k, off:off + 128], gp[:, k, off:off + 128],
                                bigeye_sb[:], op=ALU.subtract,
                            )
                    g0, gn = grp[0], len(grp)
                    nc.scalar.activation(
                        dp_sb[:, g0:g0 + gn, par, :],
                        gp[:, 0:gn, :], AF.Exp, scale=TEMP_INV,
                    )
                    if gi > 0:
                        emit_cs(groups[gi - 1])
                        emit_heat()
                    # pieces whose last-read tile was exp'd in this group
                    pending.extend((dp_sb, p) for p in due[h]
                                   if p[1] // GROUP == gi)
                    drain_reduces(2)
                emit_cs(groups[-1])
                emit_heat(HEATERS + 1)

                # evacuate CS and ship it
                cs_sb = cssbp.tile([NCLS, 512], F32, tag="cssb")
                nc.vector.tensor_copy(cs_sb[:], cs_ps[:])
                nc.sync.dma_start(cs_d[:, csl], cs_sb[:])

            drain_reduces(len(pending))
            nc.sync.dma_start(rs_d[:], rs_sb[:])

    nc.compile()
    return nc


_NC = None


def _get_program():
    global _NC
    if _NC is None:
        _NC = build_program()
    return _NC


def make_in_maps(features, labels_all):
    feats = np.asarray(features, dtype=np.float32)
    labels = np.asarray(labels_all, dtype=np.int64)
    bigeye = np.eye(128, dtype=np.float32) * BIGDIAG
    in_maps = []
    orders = []
    for b in range(B):
        order = np.argsort(labels[b], kind="stable")
        orders.append(order)
        lab = labels[b][order]
        cnt = np.bincount(lab, minlength=NCLS)
        assert cnt.max() <= MAXSEG, f"class segment {cnt.max()} > {MAXSEG}"
        f = feats[b][:, order]
        nrm = np.sqrt((f.astype(np.float64) ** 2).sum(axis=0))
        nrm = np.maximum(nrm, 1e-12)
        vhat = (f / nrm).astype(ml_dtypes.bfloat16)
        y16 = np.zeros((C, NBLK * NCLS), dtype=ml_dtypes.bfloat16)
        blk = np.arange(N) // 128
        row = np.arange(N) % 128
        y16[row, blk * NCLS + lab] = 1.0
        in_maps.append({"vhat": vhat, "y16": y16, "bigeye": bigeye})
    return in_maps, orders, labels


def finish_on_host(results, orders, labels):
    # per-point extra row-sum slots, fixed mapping
    slots_of_m = [[] for _ in range(NBLK)]
    for kind, m, h1, h2, slot in PIECES:
        slots_of_m[m].append(slot)
    losses = []
    for b in range(B):
        cs = np.asarray(results[b]["csout"], dtype=np.float64)   # [16, N]
        rs = np.asarray(results[b]["rsout"], dtype=np.float64)   # [128, RSW]
        lab = labels[b][orders[b]]
        pos = cs[lab, np.arange(N)]
        tot = cs.sum(axis=0)
        m = np.arange(N) // 128
        row = np.arange(N) % 128
        extra = np.zeros(N)
        for mm in range(NBLK):
            sel = m == mm
            if slots_of_m[mm]:
                extra[sel] = rs[row[sel]][:, slots_of_m[mm]].sum(axis=1)
        tot = tot + extra
        dev = np.log(tot) - np.log(pos)
        losses.append(dev.mean())
    return np.asarray(np.float32(np.mean(losses)))


def run(features, labels_all, **spmd_kwargs):
    nc = _get_program()
    in_maps, orders, labels = make_in_maps(features, labels_all)
    res = run_bass_kernel_spmd(nc, in_maps, list(range(B)), **spmd_kwargs)
    out = finish_on_host(res.results, orders, labels)
    return out, res


def kernel(features, labels_all):
    out, _ = run(features, labels_all)
    return out


# revision 13
# speedup vs baseline: 1.3262x; 1.3262x over previous
"""Supervised-contrastive point-cloud loss on Trainium2 (8 NeuronCores).

Inputs (full): features [8, 128, 4096] f32, labels_all [8, 4096] int32.
Sharding: data-parallel over the batch dim - core b handles cloud b.

Host prep (per cloud): sort points by label (loss is a mean over points,
so permutation-invariant), L2-normalize columns, cast to bf16.  With
sorted labels every class occupies a contiguous segment of length
<= 385 (asserted), so each point's same-class partners all lie within
3 blocks (384 rows) of its own 512-wide column chunk.

Device (per core), exploiting dp symmetry (dp = exp(10 * vhat^T vhat)):
Chunks are processed in double-chunk windows (a=2w, b=2w+1) so each
128-row stationary is loaded once and streams two back-to-back 512-col
matmuls (the PE only sustains full rate without weight swaps).
For block row m (0 .. min(4b+6, 31)):
  PE:  G(m,a), G(m,b) into a [128,2,512] PSUM tile     (skip a if m>4a+6)
  DVE: diag blocks: G -= 1e5*I  => exp underflows to 0 on the diagonal
  ACT: dp = exp(10 G) -> bf16 SBUF window [C, 32, 2, 512]
  PE:  CS[c, x] += onehot_m^T dp(m,·)  into per-chunk [16,512] PSUM
CS[c, x] = sum over rows p < 512h+896 with label c of dp[p, x], so
  positives_x = CS[label_x, x]
  totals_x    = sum_c CS[c, x]  +  sum over cols >= 512h+896 of row x
The second term is block-aligned row-direction reduces of already-
computed upper tiles (symmetry: row x of dp = column x) on DVE; two
same-window tiles reduce in one strided tensor_reduce (axis=XY).
Host tail: gather, log, mean  (O(N) numpy).
"""

import contextlib
import sys

for _p in ("/opt/trn_rl_repo",):
    if _p not in sys.path:
        sys.path.append(_p)

import numpy as np
import ml_dtypes

import concourse.bass as bass  # noqa: F401
import concourse.bacc as bacc
import concourse.tile as tile
from concourse import mybir
from concourse.bass_utils import run_bass_kernel_spmd

F32 = mybir.dt.float32
BF16 = mybir.dt.bfloat16
AF = mybir.ActivationFunctionType
ALU = mybir.AluOpType
AX = mybir.AxisListType

B, C, N = 8, 128, 4096
NCLS = 16
NBLK = N // 128          # 32 block rows
NCH = N // 512           # 8 column chunks
NW = NCH // 2            # 4 double-chunk windows
TEMP_INV = 10.0
BIGDIAG = 1.0e5
MAXSEG = 385             # max class segment length the band covers

# tiles per chunk: block rows 0 .. min(4h+6, 31)
TPC = [min(4 * h + 7, NBLK) for h in range(NCH)]


def _slot_plan():
    """Row-direction reduce pieces per block row m (hd = m//4):
    partial [384:512] of tile (m, hd+1), then full tiles (m, hd+2..7).
    Two fulls in the same window reduce in ONE tensor_reduce (axis=XY).
    piece = (kind, m, h, h2, slot); kind: 'p' partial, 't' pair, 's' single.
    """
    pieces = []
    slot = 0
    for m in range(NBLK):
        hd = m // 4
        if hd + 1 < NCH:
            pieces.append(("p", m, hd + 1, None, slot)); slot += 1
        fulls = list(range(hd + 2, NCH))
        for w in range(NW):
            hs = [h for h in fulls if h // 2 == w]
            if len(hs) == 2:
                pieces.append(("t", m, hs[0], hs[1], slot)); slot += 1
            elif len(hs) == 1:
                pieces.append(("s", m, hs[0], None, slot)); slot += 1
    return slot, pieces


NSLOT, PIECES = _slot_plan()
RSW = ((NSLOT + 3) // 4) * 4  # pad rsout width


def build_program():
    nc = bacc.Bacc("TRN2", target_bir_lowering=False, debug=False, num_devices=B)

    vhat_d = nc.dram_tensor("vhat", [C, N], BF16, kind="ExternalInput").ap()
    y16_d = nc.dram_tensor("y16", [C, NBLK * NCLS], BF16, kind="ExternalInput").ap()
    bigeye_d = nc.dram_tensor("bigeye", [128, 128], F32, kind="ExternalInput").ap()
    cs_d = nc.dram_tensor("csout", [NCLS, N], F32, kind="ExternalOutput").ap()
    rs_d = nc.dram_tensor("rsout", [128, RSW], F32, kind="ExternalOutput").ap()

    # pieces grouped by (window, ready-block): ready once exp of the
    # LAST tile the piece reads (block m, within that window) is done
    due = {}
    for kind, m, h1, h2, slot in PIECES:
        h_last = h1 if h2 is None else h2
        due.setdefault((h_last // 2, m), []).append((kind, m, h1, h2, slot))

    with tile.TileContext(nc) as tc, contextlib.ExitStack() as _stack:
        with (
            tc.tile_pool(name="const", bufs=1) as constp,
            tc.tile_pool(name="dp", bufs=2) as dpp,
            tc.tile_pool(name="cssb", bufs=2) as cssbp,
            tc.tile_pool(name="pg", bufs=3, space="PSUM") as pgp,
            tc.tile_pool(name="pcs", bufs=1, space="PSUM") as pcsp,
        ):
            # ---- constants in ----
            vhat_sb = constp.tile([C, N], BF16)
            for p in range(4):
                sl = slice(p * 1024, (p + 1) * 1024)
                nc.sync.dma_start(vhat_sb[:, sl], vhat_d[:, sl])
            y16_sb = constp.tile([C, NBLK * NCLS], BF16)
            nc.sync.dma_start(y16_sb[:], y16_d[:])
            bigeye_sb = constp.tile([128, 128], F32)
            nc.sync.dma_start(bigeye_sb[:], bigeye_d[:])

            rs_sb = constp.tile([128, RSW], F32)
            nc.gpsimd.memset(rs_sb[:], 0.0)

            # warm the Exp activation table during the DMAs
            warm = constp.tile([1, 1], F32)
            nc.gpsimd.memset(warm[:], 0.0)
            warm2 = constp.tile([1, 1], F32)
            nc.scalar.activation(warm2[:], warm[:], AF.Exp)

            pending = []          # (dp_window_tile, piece) reduce queue

            def drain_reduces(k):
                for _ in range(min(k, len(pending))):
                    dpw, (kind, m, h1, h2, slot) = pending.pop(0)
                    acc = rs_sb[:, slot:slot + 1]
                    if kind == "p":
                        nc.vector.tensor_reduce(
                            acc, dpw[:, m, h1 % 2, 384:512],
                            axis=AX.X, op=ALU.add,
                        )
                    elif kind == "s":
                        nc.vector.tensor_reduce(
                            acc, dpw[:, m, h1 % 2, :],
                            axis=AX.X, op=ALU.add,
                        )
                    else:
                        nc.vector.tensor_reduce(
                            acc, dpw[:, m, :, :],
                            axis=AX.XY, op=ALU.add,
                        )

            for w in range(NW):
                ha, hb = 2 * w, 2 * w + 1
                Ta, Tb = TPC[ha], TPC[hb]
                csa = slice(ha * 512, (ha + 1) * 512)
                csb = slice(hb * 512, (hb + 1) * 512)
                cs_a = pcsp.tile([NCLS, 512], F32, tag="csA")
                cs_b = pcsp.tile([NCLS, 512], F32, tag="csB")
                dp_sb = dpp.tile([C, NBLK, 2, 512], BF16, tag="dp")

                def emit_cs(m):
                    lhs = y16_sb[:, m * NCLS:(m + 1) * NCLS]
                    if m < Ta:
                        nc.tensor.matmul(
                            cs_a[:], lhs, dp_sb[:, m, 0, :],
                            start=(m == 0), stop=(m == Ta - 1),
                        )
                    nc.tensor.matmul(
                        cs_b[:], lhs, dp_sb[:, m, 1, :],
                        start=(m == 0), stop=(m == Tb - 1),
                    )

                for m in range(Tb):
                    gpm = pgp.tile([128, 2, 512], F32, tag="g")
                    lhs = vhat_sb[:, m * 128:(m + 1) * 128]
                    if m < Ta:
                        nc.tensor.matmul(gpm[:, 0, :], lhs, vhat_sb[:, csa],
                                         start=True, stop=True)
                    nc.tensor.matmul(gpm[:, 1, :], lhs, vhat_sb[:, csb],
                                     start=True, stop=True)
                    if m > 0:
                        emit_cs(m - 1)
                    # kill the diagonal inside the diag squares
                    if 4 * ha <= m <= 4 * ha + 3:
                        off = (m - 4 * ha) * 128
                        nc.vector.tensor_tensor(
                            gpm[:, 0, off:off + 128], gpm[:, 0, off:off + 128],
                            bigeye_sb[:], op=ALU.subtract,
                        )
                    if 4 * hb <= m <= 4 * hb + 3:
                        off = (m - 4 * hb) * 128
                        nc.vector.tensor_tensor(
                            gpm[:, 1, off:off + 128], gpm[:, 1, off:off + 128],
                            bigeye_sb[:], op=ALU.subtract,
                        )
                    if m < Ta:
                        nc.scalar.activation(
                            dp_sb[:, m, :, :], gpm[:, :, :],
                            AF.Exp, scale=TEMP_INV,
                        )
                    else:
                        nc.scalar.activation(
                            dp_sb[:, m, 1, :], gpm[:, 1, :],
                            AF.Exp, scale=TEMP_INV,
                        )
                    pending.extend((dp_sb, p) for p in due.get((w, m), []))
                    drain_reduces(2)
                emit_cs(Tb - 1)

                # evacuate CS and ship it
                cs_sba = cssbp.tile([NCLS, 512], F32, tag="cssb")
                nc.vector.tensor_copy(cs_sba[:], cs_a[:])
                nc.sync.dma_start(cs_d[:, csa], cs_sba[:])
                cs_sbb = cssbp.tile([NCLS, 512], F32, tag="cssb")
                nc.vector.tensor_copy(cs_sbb[:], cs_b[:])
                nc.sync.dma_start(cs_d[:, csb], cs_sbb[:])

            drain_reduces(len(pending))
            nc.sync.dma_start(rs_d[:], rs_sb[:])

    nc.compile()
    return nc


_NC = None


def _get_program():
    global _NC
    if _NC is None:
        _NC = build_program()
    return _NC


def make_in_maps(features, labels_all):
    feats = np.asarray(features, dtype=np.float32)
    labels = np.asarray(labels_all, dtype=np.int64)
    bigeye = np.eye(128, dtype=np.float32) * BIGDIAG
    in_maps = []
    orders = []
    for b in range(B):
        order = np.argsort(labels[b], kind="stable")
        orders.append(order)
        lab = labels[b][order]
        cnt = np.bincount(lab, minlength=NCLS)
        assert cnt.max() <= MAXSEG, f"class segment {cnt.max()} > {MAXSEG}"
        f = feats[b][:, order]
        nrm = np.sqrt((f.astype(np.float64) ** 2).sum(axis=0))
        nrm = np.maximum(nrm, 1e-12)
        vhat = (f / nrm).astype(ml_dtypes.bfloat16)
        y16 = np.zeros((C, NBLK * NCLS), dtype=ml_dtypes.bfloat16)
        blk = np.arange(N) // 128
        row = np.arange(N) % 128
        y16[row, blk * NCLS + lab] = 1.0
        in_maps.append({"vhat": vhat, "y16": y16, "bigeye": bigeye})
    return in_maps, orders, labels


def finish_on_host(results, orders, labels):
    slots_of_m = [[] for _ in range(NBLK)]
    for kind, m, h1, h2, slot in PIECES:
        slots_of_m[m].append(slot)
    losses = []
    for b in range(B):
        cs = np.asarray(results[b]["csout"], dtype=np.float64)   # [16, N]
        rs = np.asarray(results[b]["rsout"], dtype=np.float64)   # [128, RSW]
        lab = labels[b][orders[b]]
        pos = cs[lab, np.arange(N)]
        tot = cs.sum(axis=0)
        m = np.arange(N) // 128
        row = np.arange(N) % 128
        extra = np.zeros(N)
        for mm in range(NBLK):
            sel = m == mm
            if slots_of_m[mm]:
                extra[sel] = rs[row[sel]][:, slots_of_m[mm]].sum(axis=1)
        tot = tot + extra
        dev = np.log(tot) - np.log(pos)
        losses.append(dev.mean())
    return np.asarray(np.float32(np.mean(losses)))


def run(features, labels_all, **spmd_kwargs):
    nc = _get_program()
    in_maps, orders, labels = make_in_maps(features, labels_all)
    res = run_bass_kernel_spmd(nc, in_maps, list(range(B)), **spmd_kwargs)
    out = finish_on_host(res.results, orders, labels)
    return out, res


def kernel(features, labels_all):
    out, _ = run(features, labels_all)
    return out
